# revision 17
# baseline (speedup 1.0000x reference)
"""Trainium2 Bass kernel for AdditiveAttention (nn_AdditiveAttention_44564580663638).

Work-rebalanced over (batch, key-window) slots: masked keys (k >= valid_len)
contribute nothing after softmax, so they are never computed. Each of the 8
cores runs the SAME program over NS fixed-length key-window "slots"
(default [256, 192, 128] -> 576 key-cols/core instead of 1024); a runtime
solver assigns each slot a (batch, key-window) pair covering every batch's
valid keys. Cores emit per-slot unnormalized numerators N = E @ V and
denominators Z = sum_k E; the host sums per batch and divides (partial
softmax combine - exact, no rescaling needed since no rowmax is used:
|scores| <= sum|w_v| ~ 11).

Per-core pipeline per panel (qb in {0,1} x slot s, L = slot length):
  1. TensorE: A_s^T = W_q @ q_s^T [H,Q], B_s^T = W_k @ k_s^T [H,L]
  2. VectorE: pre-add s_t[:,j,:] = B^T + A^T[:,q] (4x-mode bf16)
  3. ScalarE (bottleneck): tanh over [H, G*L] groups
  4. TensorE: score rows into PSUM partitions via sliding-window one-hot
     w_v weights (32-col groups, tile_position); additive -1e6 window mask
     folds in as one extra K=1 matmul.
  5. ScalarE: E = exp(scores) from PSUM, Z from accum_out (free).
  6. TensorE: transpose E chunks, N = E^T.T @ V_s; DMA N, Z out raw.
"""

import os
import sys

for _p in ("/opt/trn_rl_repo", "/root/.axon_site/_ro/trn_rl_repo"):
    if os.path.isdir(_p) and _p not in sys.path:
        sys.path.insert(0, _p)

import math

import numpy as np
import ml_dtypes

import concourse.bass as bass
import concourse.bacc as bacc
import concourse.tile as tile
import concourse.mybir as mybir
from concourse.bass_utils import run_bass_kernel_spmd
from concourse.masks import make_identity

B, Q, K, DQ, DK, H, DV = 8, 256, 1024, 256, 256, 128, 128
P = 128
QB = Q // P      # query blocks of 128
DC = DQ // P     # contraction chunks for the projections
FP32 = mybir.dt.float32
BF16 = mybir.dt.bfloat16
BF16_NP = ml_dtypes.bfloat16
NEG = -1e6

_NC_CACHE = {}
LAST_RESULT = None


def _min_combos(vl, lengths, counts):
    """Minimal multisets of slot lengths covering vl (removing any slot drops
    below vl), as count-vectors aligned with `lengths`."""
    out = []

    def rec(i, rem, take):
        if rem <= 0:
            out.append(tuple(take + [0] * (len(lengths) - len(take))))
            return
        if i == len(lengths):
            return
        # max copies of lengths[i] usable without redundancy
        lo = 0
        hi = min(counts[i], (rem + lengths[i] - 1) // lengths[i])
        for n in range(lo, hi + 1):
            # redundancy check: only allow n copies if the last one is needed
            take.append(n)
            rec(i + 1, rem - n * lengths[i], take)
            take.pop()

    rec(0, vl, [])
    # filter non-minimal (a combo dominates another)
    minimal = []
    for c in sorted(set(out), key=lambda c: sum(n * L for n, L in zip(c, lengths))):
        if not any(all(m[i] <= c[i] for i in range(len(c))) and m != c
                   for m in minimal):
            minimal.append(c)
    return minimal


def _try_profile(vl_list, prof):
    """Backtracking assignment of batches to an 8x-replicated slot pool."""
    lengths = sorted(set(prof), reverse=True)
    pool = [8 * prof.count(L) for L in lengths]
    order = sorted(range(len(vl_list)), key=lambda b: -vl_list[b])
    combos = [_min_combos(vl_list[b], lengths, pool) for b in order]
    suffix_demand = [0] * (len(order) + 1)
    for i in range(len(order) - 1, -1, -1):
        suffix_demand[i] = suffix_demand[i + 1] + vl_list[order[i]]
    chosen = [None] * len(order)

    def rec(i, pool):
        if i == len(order):
            return True
        if sum(n * L for n, L in zip(pool, lengths)) < suffix_demand[i]:
            return False
        for c in combos[i]:
            if all(c[j] <= pool[j] for j in range(len(lengths))):
                chosen[i] = c
                if rec(i + 1, [pool[j] - c[j] for j in range(len(lengths))]):
                    return True
        return False

    if not rec(0, pool):
        return None
    assign = {b: [] for b in range(len(vl_list))}
    for i, b in enumerate(order):
        c0 = 0
        for j, L in enumerate(lengths):
            for _ in range(chosen[i][j]):
                assign[b].append((L, c0))
                c0 += L
    return assign


def _solve_slots(valid_lens):
    """Pick a per-core slot-length profile and assign (batch, window) pairs.

    Returns (profile, assign) where assign[b] = list of (slot_len, c0) windows
    covering columns [c0, c0+slot_len) of batch b, and the global pool usage
    is at most 8 slots of each profile position.
    """
    vl = [int(x) for x in valid_lens]
    total = sum(vl)
    # candidate profiles: 2-3 parts, multiples of 32, parts in [128, 512],
    # ordered by total per-core columns C (the bottleneck-engine work)
    cands = set()
    for np_ in (2, 3):
        def gen(parts):
            if len(parts) == np_:
                cands.add(tuple(parts))
                return
            for L in range(128, 513, 32):
                if not parts or L <= parts[-1]:
                    gen(parts + [L])
        gen([])
    floor = (total + 7) // 8
    for prof in sorted(cands, key=lambda p: (sum(p), len(p), -p[0])):
        if sum(prof) < floor:
            continue
        assign = _try_profile(vl, prof)
        if assign is not None:
            return prof, assign
    raise RuntimeError("no feasible slot profile")


def _build(SL):
    NS = len(SL)
    CH = [math.ceil(L / P) for L in SL]
    nc = bacc.Bacc("TRN2", target_bir_lowering=False, debug=False)
    ACT = mybir.ActivationFunctionType

    qT_ext = [nc.declare_dram_parameter(f"qT{s}", [P, DC, Q], BF16, isOutput=False)
              for s in range(NS)]
    kT_ext = [nc.declare_dram_parameter(f"kT{s}", [P, DC, SL[s]], BF16, isOutput=False)
              for s in range(NS)]
    v_ext = [nc.declare_dram_parameter(f"v{s}", [P, CH[s], DV], BF16, isOutput=False)
             for s in range(NS)]
    m_ext = [nc.declare_dram_parameter(f"m{s}", [1, SL[s]], BF16, isOutput=False)
             for s in range(NS)]
    wqT_ext = nc.declare_dram_parameter("wqT", [P, DC, H], BF16, isOutput=False)
    wkT_ext = nc.declare_dram_parameter("wkT", [P, DC, H], BF16, isOutput=False)
    wv_ext = nc.declare_dram_parameter("wv", [H, 1], BF16, isOutput=False)
    n_ext = nc.declare_dram_parameter("N", [NS, Q, DV], FP32, isOutput=True)
    z_ext = nc.declare_dram_parameter("Z", [NS, Q, 1], FP32, isOutput=True)

    # slots in per-qb emission order: biggest first (last panel = smallest)
    order = sorted(range(NS), key=lambda s: -SL[s])
    # slots whose DVE pre-add stream is slower than their tanh stream get a
    # full-panel staging tile so DVE can run arbitrarily far ahead during
    # the bigger slots' slack
    full_panel = {s for s in range(NS) if SL[s] <= 148}
    FT_BUFS = 4 if sum(SL[s] for s in range(NS) if s not in full_panel) <= 512 else 3

    with tile.TileContext(nc) as tc:
        with (
            tc.tile_pool(name="const", bufs=1) as constp,
            tc.tile_pool(name="sb", bufs=2) as sbp,
            tc.tile_pool(name="feat", bufs=3) as featp,
            tc.tile_pool(name="ps", bufs=2, space="PSUM") as psp,
            tc.tile_pool(name="ps2", bufs=2, space="PSUM") as psp2,
        ):
            # critical-path inputs for the first panel's slot, split across
            # the two HWDGE queues (SP + Activation) for parallel issue
            s0 = order[0]
            kT_sb = [constp.tile([P, DC, SL[s]], BF16, name=f"kTsb{s}")
                     for s in range(NS)]
            qT_sb = [constp.tile([P, DC, Q], BF16, name=f"qTsb{s}")
                     for s in range(NS)]
            wk_sb = constp.tile([P, DC, H], BF16)
            wq_sb = constp.tile([P, DC, H], BF16)
            nc.sync.dma_start(wk_sb[:], wkT_ext[:, :, :])
            nc.scalar.dma_start(wq_sb[:], wqT_ext[:, :, :])
            for c in range(DC):
                nc.sync.dma_start(kT_sb[s0][:, c, :], kT_ext[s0][:, c, :])
            nc.scalar.dma_start(qT_sb[s0][:], qT_ext[s0][:, :, :])

            at_sb = [constp.tile([H, Q], FP32, name=f"at{s}") for s in range(NS)]
            bt_sb = [constp.tile([H, SL[s]], BF16, name=f"bt{s}") for s in range(NS)]

            PSMAX = max(256, max(SL))

            def emit_proj(s, head=False):
                bt_ps = psp.tile([H, PSMAX], FP32, tag="pj", bufs=1)
                for c in range(DC):
                    nc.tensor.matmul(
                        bt_ps[:, 0:SL[s]], wk_sb[:, c, :], kT_sb[s][:, c, :],
                        start=(c == 0), stop=(c == DC - 1),
                    )
                if head:
                    # ScalarE is idle pre-tanh: do the bt copy there so the
                    # DVE can do the at copy concurrently
                    nc.scalar.copy(bt_sb[s][:], bt_ps[:, 0:SL[s]])
                else:
                    nc.vector.tensor_copy(bt_sb[s][:], bt_ps[:, 0:SL[s]])
                at_ps = psp.tile([H, PSMAX], FP32, tag="pj", bufs=1)
                for c in range(DC):
                    nc.tensor.matmul(
                        at_ps[:, 0:Q], wq_sb[:, c, :], qT_sb[s][:, c, :],
                        start=(c == 0), stop=(c == DC - 1),
                    )
                nc.vector.tensor_copy(at_sb[s][:], at_ps[:, 0:Q])

            emit_proj(s0, head=True)

            # small constants up front (needed by panel 0's score matmuls)
            m_sb = [constp.tile([1, SL[s]], BF16, name=f"msb{s}") for s in range(NS)]
            for s in range(NS):
                nc.sync.dma_start(m_sb[s][:], m_ext[s][:, :])
            wv_sb = constp.tile([H, 1], BF16)
            nc.sync.dma_start(wv_sb[:], wv_ext[:, :])
            ones_sb = constp.tile([1, P], BF16)
            nc.vector.memset(ones_sb[:], 1.0)
            ident = constp.tile([P, P], BF16)
            make_identity(nc, ident[:])
            # sliding-window one-hot w_v: wvstrip[:, 32:33] = w_v; then
            # wvstrip[:, 32-r : 64-r] is w_v (x) e_r^T over a 32-col group.
            wvstrip = constp.tile([H, 65], BF16)
            nc.vector.memset(wvstrip[:], 0.0)
            nc.vector.tensor_copy(wvstrip[:, 32:33], wv_sb[:, :])

            v_sb = [constp.tile([P, CH[s], DV], BF16, name=f"vsb{s}")
                    for s in range(NS)]

            def emit_bulk_dmas():
                for s in range(NS):
                    if s != s0:
                        for c in range(DC):
                            nc.sync.dma_start(kT_sb[s][:, c, :], kT_ext[s][:, c, :])
                        nc.sync.dma_start(qT_sb[s][:], qT_ext[s][:, :, :])
                for s in range(NS):
                    nc.sync.dma_start(v_sb[s][:], v_ext[s][:, :, :])

            def emit_epilogue(qb, s, sc_ps):
                L = SL[s]
                e_sb = sbp.tile([P, L], BF16, tag=f"e{s}")
                z_sb = sbp.tile([P, 1], FP32, tag=f"z{s}")
                nc.scalar.activation(
                    e_sb[:], sc_ps[:, 0:L], ACT.Exp, accum_out=z_sb[:],
                )
                o_ps = psp2.tile([P, DV], FP32, tag="o_ps")
                for kc in range(CH[s]):
                    pw = min(P, L - kc * P)
                    tp_ps = psp2.tile([P, P], BF16, tag="tp")
                    nc.tensor.transpose(
                        tp_ps[0:pw, :], e_sb[:, kc * P : kc * P + pw], ident[:],
                    )
                    ptc = sbp.tile([P, P], BF16, tag="pt")
                    nc.vector.tensor_copy(ptc[0:pw, :], tp_ps[0:pw, :])
                    nc.tensor.matmul(
                        o_ps[:], ptc[0:pw, :], v_sb[s][0:pw, kc, :],
                        start=(kc == 0), stop=(kc == CH[s] - 1),
                    )
                o_sb = sbp.tile([P, DV], FP32, tag="o_sb")
                nc.vector.tensor_copy(o_sb[:], o_ps[:])
                nc.sync.dma_start(n_ext[s, qb * P : (qb + 1) * P, :], o_sb[:])
                nc.sync.dma_start(z_ext[s, qb * P : (qb + 1) * P, :], z_sb[:])

            # graded group sizes: small at kernel start (fast pipeline fill)
            # and end (short epilogue lag), large interior (amortize ScalarE
            # per-instruction overhead). Full-panel slots use 32-query tanh
            # groups (their pre-adds are staged ahead in a panel-sized tile).
            ramp_up = [4, 12]
            ramp_dn = [8, 8]
            panels = [(qb, s) for qb in range(QB) for s in order]
            pending = None
            for pi, (qb, s) in enumerate(panels):
                L = SL[s]
                gmax = 32 if s in full_panel else 16
                if pi == 0:
                    ramp = ramp_up + ([16] if gmax == 32 else [])
                    sizes = ramp + [gmax] * ((P - sum(ramp)) // gmax)
                elif pi == len(panels) - 1:
                    ramp = ([16] if gmax == 32 else []) + ramp_dn
                    sizes = [gmax] * ((P - sum(ramp)) // gmax) + ramp
                else:
                    sizes = [gmax] * (P // gmax)
                assert sum(sizes) == P
                sc_ps = psp.tile([P, PSMAX], FP32, tag="sc", bufs=3)
                if s in full_panel:
                    # stage the whole panel's pre-adds in one tile on the
                    # otherwise-idle GpSimd engine: it streams these during
                    # earlier slots' phases without contending with the DVE
                    s_tp = featp.tile([H, P, L], BF16, tag=f"sp{s}", bufs=1)
                    for q in range(P):
                        nc.gpsimd.tensor_scalar_add(
                            s_tp[:, q, :], bt_sb[s][:],
                            at_sb[s][:, qb * P + q : qb * P + q + 1],
                        )
                q0 = 0
                for gi, gsz in enumerate(sizes):
                    if s in full_panel:
                        ft = featp.tile([H, gmax, L], BF16, tag=f"ft{s}", bufs=2)
                        nc.scalar.activation(
                            ft[:, 0:gsz, :], s_tp[:, q0 : q0 + gsz, :], ACT.Tanh
                        )
                    else:
                        s_t = featp.tile([H, gmax, L], BF16, tag=f"s{s}", bufs=2)
                        for j in range(gsz):
                            qg = qb * P + q0 + j
                            nc.vector.tensor_scalar_add(
                                s_t[:, j, :], bt_sb[s][:], at_sb[s][:, qg : qg + 1]
                            )
                        ft = featp.tile([H, gmax, L], BF16, tag=f"ft{s}", bufs=FT_BUFS)
                        nc.scalar.activation(
                            ft[:, 0:gsz, :], s_t[:, 0:gsz, :], ACT.Tanh
                        )
                    # score rows land in their PSUM partition via sliding
                    # one-hot weights; 32-col weight loads are 4x cheaper.
                    for j in range(gsz):
                        qi = q0 + j
                        cg, r = qi // 32, qi % 32
                        nc.tensor.matmul(
                            sc_ps[cg * 32 : (cg + 1) * 32, 0:L],
                            wvstrip[:, 32 - r : 64 - r],
                            ft[:, j, :],
                            start=(r == 0), stop=(qi == P - 1),
                            skip_group_check=True,
                            tile_position=(0, cg * 32),
                        )
                    if q0 <= 96 < q0 + gsz:
                        # additive -1e6 window mask on every row; emitted
                        # after every 32-row col-group region has started.
                        nc.tensor.matmul(
                            sc_ps[:, 0:L], ones_sb[:], m_sb[s][:, :],
                            start=False, stop=False, skip_group_check=True,
                        )
                    q0 += gsz
                    if pi == 0:
                        if gi == 0:
                            emit_bulk_dmas()
                        elif gi >= 2 and gi - 2 < len(order) - 1:
                            emit_proj(order[gi - 1])
                    # previous panel's epilogue rides behind this panel's
                    # first group so its DVE burst never stalls the pre-adds
                    if gi == 0 and pending is not None:
                        emit_epilogue(*pending)
                        pending = None
                pending = (qb, s, sc_ps)
            emit_epilogue(*pending)

    nc.compile()
    return nc


def _get_nc(SL):
    if SL not in _NC_CACHE:
        _NC_CACHE[SL] = _build(SL)
    return _NC_CACHE[SL]


def _pack(mat):
    # [C*P, F] -> [P, C, F]: partition-major so each SBUF partition's
    # data is one contiguous DRAM run (fast, few DMA descriptors)
    cp, f = mat.shape
    c = cp // P
    return np.ascontiguousarray(
        mat.reshape(c, P, f).transpose(1, 0, 2)
    ).astype(BF16_NP)


def kernel(queries, keys, values, valid_lens, W_q, W_k, w_v):
    global LAST_RESULT
    queries = np.asarray(queries, dtype=np.float32)
    keys = np.asarray(keys, dtype=np.float32)
    values = np.asarray(values, dtype=np.float32)
    valid_lens = np.asarray(valid_lens, dtype=np.int32)
    W_q = np.asarray(W_q, dtype=np.float32)
    W_k = np.asarray(W_k, dtype=np.float32)
    w_v = np.asarray(w_v, dtype=np.float32)

    SL, assign = _solve_slots(valid_lens)
    NS = len(SL)
    CH = [math.ceil(L / P) for L in SL]

    # distribute each batch's windows to per-core slot positions: for each
    # profile length, hand its (batch, window) list out across the 8 cores
    # and the slot positions having that length
    core_slots = [[None] * NS for _ in range(B)]
    pos_by_len = {}
    for si, L in enumerate(SL):
        pos_by_len.setdefault(L, []).append(si)
    for L, positions in pos_by_len.items():
        jobs = []
        for b in range(B):
            jobs += [(b, c0) for (Lw, c0) in assign[b] if Lw == L]
        assert len(jobs) <= B * len(positions)
        for i, job in enumerate(jobs):
            core_slots[i % B][positions[i // B]] = job

    wqT = _pack(W_q.T)                                        # [P, DC, H]
    wkT = _pack(W_k.T)                                        # [P, DC, H]
    wvc = np.ascontiguousarray(w_v[:, None]).astype(BF16_NP)  # [H, 1]

    qT_packed = [_pack(queries[b].T) for b in range(B)]
    keysT = [keys[b].T for b in range(B)]                     # [DK, K]

    in_maps = []
    for core in range(B):
        im = {"wqT": wqT, "wkT": wkT, "wv": wvc,
              "N": np.zeros((NS, Q, DV), dtype=np.float32),
              "Z": np.zeros((NS, Q, 1), dtype=np.float32)}
        for si, L in enumerate(SL):
            job = core_slots[core][si]
            if job is None:
                im[f"qT{si}"] = np.zeros((P, DC, Q), dtype=BF16_NP)
                im[f"kT{si}"] = np.zeros((P, DC, L), dtype=BF16_NP)
                im[f"v{si}"] = np.zeros((P, CH[si], DV), dtype=BF16_NP)
                im[f"m{si}"] = np.full((1, L), NEG, dtype=np.float32).astype(BF16_NP)
                continue
            b, c0 = job
            vlb = int(valid_lens[b])
            nreal = max(0, min(c0 + L, K) - c0)
            kw = np.zeros((DK, L), dtype=np.float32)
            kw[:, :nreal] = keysT[b][:, c0 : c0 + nreal]
            vw = np.zeros((CH[si] * P, DV), dtype=np.float32)
            vw[:nreal] = values[b][c0 : c0 + nreal]
            idx = c0 + np.arange(L)
            mrow = np.where(idx < vlb, 0.0, NEG).astype(np.float32)
            im[f"qT{si}"] = qT_packed[b]
            im[f"kT{si}"] = _pack(kw)
            im[f"v{si}"] = _pack(vw)
            im[f"m{si}"] = mrow[None, :].astype(BF16_NP)
        in_maps.append(im)

    nc = _get_nc(SL)
    trace = bool(int(os.environ.get("KERNEL_TRACE", "0")))
    res = run_bass_kernel_spmd(nc, in_maps, core_ids=list(range(B)), trace=trace)
    LAST_RESULT = res

    nacc = np.zeros((B, Q, DV), dtype=np.float32)
    zacc = np.zeros((B, Q), dtype=np.float32)
    for core in range(B):
        rN = np.asarray(res.results[core]["N"], dtype=np.float32)
        rZ = np.asarray(res.results[core]["Z"], dtype=np.float32)
        for si in range(NS):
            job = core_slots[core][si]
            if job is None:
                continue
            b = job[0]
            nacc[b] += rN[si]
            zacc[b] += rZ[si, :, 0]
    return nacc / zacc[..., None]


# revision 19
# speedup vs baseline: 4.0122x; 4.0122x over previous
"""Trainium2 Bass kernel for AdditiveAttention (nn_AdditiveAttention_44564580663638).

Work-rebalanced over (batch, key-window) slots: masked keys (k >= valid_len)
contribute nothing after softmax, so they are never computed. Each of the 8
cores runs the SAME program over NS fixed-length key-window "slots"
(default [256, 192, 128] -> 576 key-cols/core instead of 1024); a runtime
solver assigns each slot a (batch, key-window) pair covering every batch's
valid keys. Cores emit per-slot unnormalized numerators N = E @ V and
denominators Z = sum_k E; the host sums per batch and divides (partial
softmax combine - exact, no rescaling needed since no rowmax is used:
|scores| <= sum|w_v| ~ 11).

Per-core pipeline per panel (qb in {0,1} x slot s, L = slot length):
  1. TensorE: A_s^T = W_q @ q_s^T [H,Q], B_s^T = W_k @ k_s^T [H,L]
  2. VectorE: pre-add s_t[:,j,:] = B^T + A^T[:,q] (4x-mode bf16)
  3. ScalarE (bottleneck): tanh over [H, G*L] groups
  4. TensorE: score rows into PSUM partitions via sliding-window one-hot
     w_v weights (32-col groups, tile_position); additive -1e6 window mask
     folds in as one extra K=1 matmul.
  5. ScalarE: E = exp(scores) from PSUM, Z from accum_out (free).
  6. TensorE: transpose E chunks, N = E^T.T @ V_s; DMA N, Z out raw.
"""

import os
import sys

for _p in ("/opt/trn_rl_repo", "/root/.axon_site/_ro/trn_rl_repo"):
    if os.path.isdir(_p) and _p not in sys.path:
        sys.path.insert(0, _p)

import math

import numpy as np
import ml_dtypes

import concourse.bass as bass
import concourse.bacc as bacc
import concourse.tile as tile
import concourse.mybir as mybir
from concourse.bass_utils import run_bass_kernel_spmd
from concourse.masks import make_identity

B, Q, K, DQ, DK, H, DV = 8, 256, 1024, 256, 256, 128, 128
P = 128
QB = Q // P      # query blocks of 128
DC = DQ // P     # contraction chunks for the projections
FP32 = mybir.dt.float32
BF16 = mybir.dt.bfloat16
BF16_NP = ml_dtypes.bfloat16
NEG = -1e6

_NC_CACHE = {}
LAST_RESULT = None


def _min_combos(vl, lengths, counts):
    """Minimal multisets of slot lengths covering vl (removing any slot drops
    below vl), as count-vectors aligned with `lengths`."""
    out = []

    def rec(i, rem, take):
        if rem <= 0:
            out.append(tuple(take + [0] * (len(lengths) - len(take))))
            return
        if i == len(lengths):
            return
        # max copies of lengths[i] usable without redundancy
        lo = 0
        hi = min(counts[i], (rem + lengths[i] - 1) // lengths[i])
        for n in range(lo, hi + 1):
            # redundancy check: only allow n copies if the last one is needed
            take.append(n)
            rec(i + 1, rem - n * lengths[i], take)
            take.pop()

    rec(0, vl, [])
    # filter non-minimal (a combo dominates another)
    minimal = []
    for c in sorted(set(out), key=lambda c: sum(n * L for n, L in zip(c, lengths))):
        if not any(all(m[i] <= c[i] for i in range(len(c))) and m != c
                   for m in minimal):
            minimal.append(c)
    return minimal


def _try_profile(vl_list, prof):
    """Backtracking assignment of batches to an 8x-replicated slot pool."""
    lengths = sorted(set(prof), reverse=True)
    pool = [8 * prof.count(L) for L in lengths]
    order = sorted(range(len(vl_list)), key=lambda b: -vl_list[b])
    combos = [_min_combos(vl_list[b], lengths, pool) for b in order]
    suffix_demand = [0] * (len(order) + 1)
    for i in range(len(order) - 1, -1, -1):
        suffix_demand[i] = suffix_demand[i + 1] + vl_list[order[i]]
    chosen = [None] * len(order)

    def rec(i, pool):
        if i == len(order):
            return True
        if sum(n * L for n, L in zip(pool, lengths)) < suffix_demand[i]:
            return False
        for c in combos[i]:
            if all(c[j] <= pool[j] for j in range(len(lengths))):
                chosen[i] = c
                if rec(i + 1, [pool[j] - c[j] for j in range(len(lengths))]):
                    return True
        return False

    if not rec(0, pool):
        return None
    assign = {b: [] for b in range(len(vl_list))}
    for i, b in enumerate(order):
        c0 = 0
        for j, L in enumerate(lengths):
            for _ in range(chosen[i][j]):
                assign[b].append((L, c0))
                c0 += L
    return assign


def _solve_slots(valid_lens):
    """Pick a per-core slot-length profile and assign (batch, window) pairs.

    Returns (profile, assign) where assign[b] = list of (slot_len, c0) windows
    covering columns [c0, c0+slot_len) of batch b, and the global pool usage
    is at most 8 slots of each profile position.
    """
    vl = [int(x) for x in valid_lens]
    total = sum(vl)
    # candidate profiles: 2-3 parts, multiples of 32, parts in [128, 512],
    # ordered by total per-core columns C (the bottleneck-engine work)
    cands = set()
    for np_ in (2, 3):
        def gen(parts):
            if len(parts) == np_:
                cands.add(tuple(parts))
                return
            for L in range(128, 513, 32):
                if not parts or L <= parts[-1]:
                    gen(parts + [L])
        gen([])
    floor = (total + 7) // 8
    for prof in sorted(cands, key=lambda p: (sum(p), len(p), -p[0])):
        if sum(prof) < floor:
            continue
        assign = _try_profile(vl, prof)
        if assign is not None:
            return prof, assign
    raise RuntimeError("no feasible slot profile")


def _build(SL):
    NS = len(SL)
    CH = [math.ceil(L / P) for L in SL]
    nc = bacc.Bacc("TRN2", target_bir_lowering=False, debug=False)
    ACT = mybir.ActivationFunctionType

    qT_ext = [nc.declare_dram_parameter(f"qT{s}", [P, DC, Q], BF16, isOutput=False)
              for s in range(NS)]
    kT_ext = [nc.declare_dram_parameter(f"kT{s}", [P, DC, SL[s]], BF16, isOutput=False)
              for s in range(NS)]
    v_ext = [nc.declare_dram_parameter(f"v{s}", [P, CH[s], DV], BF16, isOutput=False)
             for s in range(NS)]
    m_ext = [nc.declare_dram_parameter(f"m{s}", [1, SL[s]], BF16, isOutput=False)
             for s in range(NS)]
    wqT_ext = nc.declare_dram_parameter("wqT", [P, DC, H], BF16, isOutput=False)
    wkT_ext = nc.declare_dram_parameter("wkT", [P, DC, H], BF16, isOutput=False)
    wv_ext = nc.declare_dram_parameter("wv", [H, 1], BF16, isOutput=False)
    n_ext = nc.declare_dram_parameter("N", [NS, Q, DV], FP32, isOutput=True)
    z_ext = nc.declare_dram_parameter("Z", [NS, Q, 1], FP32, isOutput=True)

    # slots in per-qb emission order: biggest first (last panel = smallest)
    order = sorted(range(NS), key=lambda s: -SL[s])
    # slots whose DVE pre-add stream is slower than their tanh stream get a
    # deeper s_t rotation so the DVE can run a few groups ahead during the
    # bigger slots' slack (the buffer count caps how much the scheduler may
    # front-load, which big-slot slack must absorb)
    deep = {s for s in range(NS) if SL[s] <= 148}
    FT_BUFS = 4 if sum(SL) <= 832 else 3

    with tile.TileContext(nc) as tc:
        with (
            tc.tile_pool(name="const", bufs=1) as constp,
            tc.tile_pool(name="sb", bufs=2) as sbp,
            tc.tile_pool(name="feat", bufs=3) as featp,
            tc.tile_pool(name="ps", bufs=2, space="PSUM") as psp,
            tc.tile_pool(name="ps2", bufs=2, space="PSUM") as psp2,
        ):
            # critical-path inputs for the first panel's slot, split across
            # the two HWDGE queues (SP + Activation) for parallel issue
            s0 = order[0]
            kT_sb = [constp.tile([P, DC, SL[s]], BF16, name=f"kTsb{s}")
                     for s in range(NS)]
            qT_sb = [constp.tile([P, DC, Q], BF16, name=f"qTsb{s}")
                     for s in range(NS)]
            wk_sb = constp.tile([P, DC, H], BF16)
            wq_sb = constp.tile([P, DC, H], BF16)
            nc.sync.dma_start(wk_sb[:], wkT_ext[:, :, :])
            nc.scalar.dma_start(wq_sb[:], wqT_ext[:, :, :])
            for c in range(DC):
                nc.sync.dma_start(kT_sb[s0][:, c, :], kT_ext[s0][:, c, :])
            nc.scalar.dma_start(qT_sb[s0][:], qT_ext[s0][:, :, :])

            at_sb = [constp.tile([H, Q], FP32, name=f"at{s}") for s in range(NS)]
            bt_sb = [constp.tile([H, SL[s]], BF16, name=f"bt{s}") for s in range(NS)]

            PSMAX = max(256, max(SL))

            def emit_proj(s, head=False):
                bt_ps = psp.tile([H, PSMAX], FP32, tag="pj", bufs=1)
                for c in range(DC):
                    nc.tensor.matmul(
                        bt_ps[:, 0:SL[s]], wk_sb[:, c, :], kT_sb[s][:, c, :],
                        start=(c == 0), stop=(c == DC - 1),
                    )
                if head:
                    # ScalarE is idle pre-tanh: do the bt copy there so the
                    # DVE can do the at copy concurrently
                    nc.scalar.copy(bt_sb[s][:], bt_ps[:, 0:SL[s]])
                else:
                    nc.vector.tensor_copy(bt_sb[s][:], bt_ps[:, 0:SL[s]])
                at_ps = psp.tile([H, PSMAX], FP32, tag="pj", bufs=1)
                for c in range(DC):
                    nc.tensor.matmul(
                        at_ps[:, 0:Q], wq_sb[:, c, :], qT_sb[s][:, c, :],
                        start=(c == 0), stop=(c == DC - 1),
                    )
                nc.vector.tensor_copy(at_sb[s][:], at_ps[:, 0:Q])

            emit_proj(s0, head=True)

            # small constants up front (needed by panel 0's score matmuls)
            m_sb = [constp.tile([1, SL[s]], BF16, name=f"msb{s}") for s in range(NS)]
            for s in range(NS):
                nc.sync.dma_start(m_sb[s][:], m_ext[s][:, :])
            wv_sb = constp.tile([H, 1], BF16)
            nc.sync.dma_start(wv_sb[:], wv_ext[:, :])
            ones_sb = constp.tile([1, P], BF16)
            nc.vector.memset(ones_sb[:], 1.0)
            ident = constp.tile([P, P], BF16)
            make_identity(nc, ident[:])
            # sliding-window one-hot w_v: wvstrip[:, 32:33] = w_v; then
            # wvstrip[:, 32-r : 64-r] is w_v (x) e_r^T over a 32-col group.
            wvstrip = constp.tile([H, 65], BF16)
            nc.vector.memset(wvstrip[:], 0.0)
            nc.vector.tensor_copy(wvstrip[:, 32:33], wv_sb[:, :])

            v_sb = [constp.tile([P, CH[s], DV], BF16, name=f"vsb{s}")
                    for s in range(NS)]

            def emit_bulk_dmas():
                for s in range(NS):
                    if s != s0:
                        for c in range(DC):
                            nc.sync.dma_start(kT_sb[s][:, c, :], kT_ext[s][:, c, :])
                        nc.sync.dma_start(qT_sb[s][:], qT_ext[s][:, :, :])
                for s in range(NS):
                    nc.sync.dma_start(v_sb[s][:], v_ext[s][:, :, :])

            def emit_epilogue(qb, s, sc_ps):
                L = SL[s]
                e_sb = sbp.tile([P, L], BF16, tag=f"e{s}")
                z_sb = sbp.tile([P, 1], FP32, tag=f"z{s}")
                nc.scalar.activation(
                    e_sb[:], sc_ps[:, 0:L], ACT.Exp, accum_out=z_sb[:],
                )
                o_ps = psp2.tile([P, DV], FP32, tag="o_ps")
                for kc in range(CH[s]):
                    pw = min(P, L - kc * P)
                    tp_ps = psp2.tile([P, P], BF16, tag="tp")
                    nc.tensor.transpose(
                        tp_ps[0:pw, :], e_sb[:, kc * P : kc * P + pw], ident[:],
                    )
                    ptc = sbp.tile([P, P], BF16, tag="pt")
                    nc.vector.tensor_copy(ptc[0:pw, :], tp_ps[0:pw, :])
                    nc.tensor.matmul(
                        o_ps[:], ptc[0:pw, :], v_sb[s][0:pw, kc, :],
                        start=(kc == 0), stop=(kc == CH[s] - 1),
                    )
                o_sb = sbp.tile([P, DV], FP32, tag="o_sb")
                nc.vector.tensor_copy(o_sb[:], o_ps[:])
                nc.sync.dma_start(n_ext[s, qb * P : (qb + 1) * P, :], o_sb[:])
                nc.sync.dma_start(z_ext[s, qb * P : (qb + 1) * P, :], z_sb[:])

            # graded group sizes: small at kernel start (fast pipeline fill)
            # and end (short epilogue lag), large interior (amortize ScalarE
            # per-instruction overhead). Full-panel slots use 32-query tanh
            # groups (their pre-adds are staged ahead in a panel-sized tile).
            ramp_up = [4, 12]
            ramp_dn = [8, 8]
            panels = [(qb, s) for qb in range(QB) for s in order]
            pending = None
            for pi, (qb, s) in enumerate(panels):
                L = SL[s]
                gmax = 16
                if pi == 0:
                    sizes = ramp_up + [gmax] * ((P - sum(ramp_up)) // gmax)
                elif pi == len(panels) - 1:
                    sizes = [gmax] * ((P - sum(ramp_dn)) // gmax) + ramp_dn
                else:
                    sizes = [gmax] * (P // gmax)
                assert sum(sizes) == P
                sc_ps = psp.tile([P, PSMAX], FP32, tag="sc", bufs=3)
                q0 = 0
                for gi, gsz in enumerate(sizes):
                    s_t = featp.tile(
                        [H, gmax, L], BF16, tag=f"s{s}",
                        bufs=4 if s in deep else 2,
                    )
                    for j in range(gsz):
                        qg = qb * P + q0 + j
                        nc.vector.tensor_scalar_add(
                            s_t[:, j, :], bt_sb[s][:], at_sb[s][:, qg : qg + 1]
                        )
                    ft = featp.tile([H, gmax, L], BF16, tag=f"ft{s}", bufs=FT_BUFS)
                    nc.scalar.activation(
                        ft[:, 0:gsz, :], s_t[:, 0:gsz, :], ACT.Tanh
                    )
                    # score rows land in their PSUM partition via sliding
                    # one-hot weights; 32-col weight loads are 4x cheaper.
                    for j in range(gsz):
                        qi = q0 + j
                        cg, r = qi // 32, qi % 32
                        nc.tensor.matmul(
                            sc_ps[cg * 32 : (cg + 1) * 32, 0:L],
                            wvstrip[:, 32 - r : 64 - r],
                            ft[:, j, :],
                            start=(r == 0), stop=(qi == P - 1),
                            skip_group_check=True,
                            tile_position=(0, cg * 32),
                        )
                    if q0 <= 96 < q0 + gsz:
                        # additive -1e6 window mask on every row; emitted
                        # after every 32-row col-group region has started.
                        nc.tensor.matmul(
                            sc_ps[:, 0:L], ones_sb[:], m_sb[s][:, :],
                            start=False, stop=False, skip_group_check=True,
                        )
                    q0 += gsz
                    if pi == 0:
                        if gi == 0:
                            emit_bulk_dmas()
                        elif gi >= 2 and gi - 2 < len(order) - 1:
                            emit_proj(order[gi - 1])
                    # previous panel's epilogue rides behind this panel's
                    # first group so its DVE burst never stalls the pre-adds
                    if gi == 0 and pending is not None:
                        emit_epilogue(*pending)
                        pending = None
                pending = (qb, s, sc_ps)
            emit_epilogue(*pending)

    nc.compile()
    return nc


def _get_nc(SL):
    if SL not in _NC_CACHE:
        _NC_CACHE[SL] = _build(SL)
    return _NC_CACHE[SL]


def _pack(mat):
    # [C*P, F] -> [P, C, F]: partition-major so each SBUF partition's
    # data is one contiguous DRAM run (fast, few DMA descriptors)
    cp, f = mat.shape
    c = cp // P
    return np.ascontiguousarray(
        mat.reshape(c, P, f).transpose(1, 0, 2)
    ).astype(BF16_NP)


def kernel(queries, keys, values, valid_lens, W_q, W_k, w_v):
    global LAST_RESULT
    queries = np.asarray(queries, dtype=np.float32)
    keys = np.asarray(keys, dtype=np.float32)
    values = np.asarray(values, dtype=np.float32)
    valid_lens = np.asarray(valid_lens, dtype=np.int32)
    W_q = np.asarray(W_q, dtype=np.float32)
    W_k = np.asarray(W_k, dtype=np.float32)
    w_v = np.asarray(w_v, dtype=np.float32)

    SL, assign = _solve_slots(valid_lens)
    NS = len(SL)
    CH = [math.ceil(L / P) for L in SL]

    # distribute each batch's windows to per-core slot positions: for each
    # profile length, hand its (batch, window) list out across the 8 cores
    # and the slot positions having that length
    core_slots = [[None] * NS for _ in range(B)]
    pos_by_len = {}
    for si, L in enumerate(SL):
        pos_by_len.setdefault(L, []).append(si)
    for L, positions in pos_by_len.items():
        jobs = []
        for b in range(B):
            jobs += [(b, c0) for (Lw, c0) in assign[b] if Lw == L]
        assert len(jobs) <= B * len(positions)
        for i, job in enumerate(jobs):
            core_slots[i % B][positions[i // B]] = job

    wqT = _pack(W_q.T)                                        # [P, DC, H]
    wkT = _pack(W_k.T)                                        # [P, DC, H]
    wvc = np.ascontiguousarray(w_v[:, None]).astype(BF16_NP)  # [H, 1]

    qT_packed = [_pack(queries[b].T) for b in range(B)]
    keysT = [keys[b].T for b in range(B)]                     # [DK, K]

    in_maps = []
    for core in range(B):
        im = {"wqT": wqT, "wkT": wkT, "wv": wvc,
              "N": np.zeros((NS, Q, DV), dtype=np.float32),
              "Z": np.zeros((NS, Q, 1), dtype=np.float32)}
        for si, L in enumerate(SL):
            job = core_slots[core][si]
            if job is None:
                im[f"qT{si}"] = np.zeros((P, DC, Q), dtype=BF16_NP)
                im[f"kT{si}"] = np.zeros((P, DC, L), dtype=BF16_NP)
                im[f"v{si}"] = np.zeros((P, CH[si], DV), dtype=BF16_NP)
                im[f"m{si}"] = np.full((1, L), NEG, dtype=np.float32).astype(BF16_NP)
                continue
            b, c0 = job
            vlb = int(valid_lens[b])
            nreal = max(0, min(c0 + L, K) - c0)
            kw = np.zeros((DK, L), dtype=np.float32)
            kw[:, :nreal] = keysT[b][:, c0 : c0 + nreal]
            vw = np.zeros((CH[si] * P, DV), dtype=np.float32)
            vw[:nreal] = values[b][c0 : c0 + nreal]
            idx = c0 + np.arange(L)
            mrow = np.where(idx < vlb, 0.0, NEG).astype(np.float32)
            im[f"qT{si}"] = qT_packed[b]
            im[f"kT{si}"] = _pack(kw)
            im[f"v{si}"] = _pack(vw)
            im[f"m{si}"] = mrow[None, :].astype(BF16_NP)
        in_maps.append(im)

    nc = _get_nc(SL)
    trace = bool(int(os.environ.get("KERNEL_TRACE", "0")))
    res = run_bass_kernel_spmd(nc, in_maps, core_ids=list(range(B)), trace=trace)
    LAST_RESULT = res

    nacc = np.zeros((B, Q, DV), dtype=np.float32)
    zacc = np.zeros((B, Q), dtype=np.float32)
    for core in range(B):
        rN = np.asarray(res.results[core]["N"], dtype=np.float32)
        rZ = np.asarray(res.results[core]["Z"], dtype=np.float32)
        for si in range(NS):
            job = core_slots[core][si]
            if job is None:
                continue
            b = job[0]
            nacc[b] += rN[si]
            zacc[b] += rZ[si, :, 0]
    return nacc / zacc[..., None]
